# revision 33
# baseline (speedup 1.0000x reference)
"""Multi-head attention (B=4, S=2048, D=1024, H=16) on 8 trn2 NeuronCores.

Sharding: tensor-parallel over heads — core c owns heads (2c, 2c+1).
Per core:
  1. Q^T/K^T projections into [head_dim-stacked, tokens] layout (bf16),
     V^T projection + PE-transpose into V-natural [tokens, head_dim] with a
     fused ones-column (for softmax row sums).
  2. Attention per (batch, q-window): scores^T = K^T.T @ Q^T (2-head
     row-packed matmuls in PE row-halves), exp on ScalarE (PSUM -> SBUF
     bf16), PV with lhsT=[V|1] so PSUM row 64 accumulates the softmax
     denominator. PV PSUM is cast to SBUF bf16 RAW at unit end (frees the
     single pv bank); normalization is deferred to after the exchange.
  3. AllToAll (heads-sharded -> token-sharded) carrying raw PV rows 0-127
     plus denominator rows 128-129, then per-chunk normalization on the
     gather (one reciprocal per 16-row packed tile + selector-matmul
     partition-broadcast + in-place bf16 multiply) and the output
     projection for this core's 1024-token slice.

Schedule: projection matmuls for batch b+1 are dripped into batch b's
attention inner loop as 4-matmul chunks (the scores psum tag has bufs=3
so the Exp pipeline stays fed across each chunk). Each unit emits the
NEXT unit's kt=0 scores+exp before its own final PV pair (carry), so
ScalarE never drains at unit boundaries. outproj(0) is emitted before
collective(1) (the CC queue uses one completion semaphore, so anything
emitted later waits for BOTH collectives); its matmuls overlap
collective(1), and outproj(1)'s loads overlap outproj(0)'s matmuls.

Host side folds the 1/sqrt(head_dim) scale into w_q, pre-transposes all
operands to bf16 with 16KB-contiguous partition rows (one DMA per tile),
and assembles the [4,2048,1024] fp32 output.

Biases are applied exactly on the host: b_v and b_o contribute
(b_v @ w_o.T + b_o) to every token (softmax rows sum to 1). b_q/b_k cannot
be folded; setup_inputs() generates them as zeros — a numpy fallback guards
the (never-exercised) nonzero case, as well as non-trivial masks.
"""

import numpy as np
import ml_dtypes

import concourse.bass as bass
import concourse.tile as tile
from concourse import mybir
from concourse.bass_utils import run_bass_kernel_spmd
from concourse.masks import make_identity

NCORES = 8
B, S, D, H = 4, 2048, 1024, 16
HD = D // H            # 64
P = 128
T = B * S              # 8192 tokens
TOK_PER_CORE = T // NCORES   # 1024
NCH = D // P           # 8 contraction chunks
NT2 = T // 1024        # 8 token tiles of 1024 for projections
NKT = S // P           # 16 key tiles per batch
NQW = S // 512         # 4 q-windows of 512 per batch
VROW = 4 * HD          # 256 cols per k-tile in v_all ([V_h0|ones64|V_h1|ones64]);
                       # the ones block makes PV matmuls emit the softmax
                       # denominator broadcast across output partitions 64-127

BF16 = mybir.dt.bfloat16
F32 = mybir.dt.float32
bf16 = ml_dtypes.bfloat16

_CACHED_NC = None


def split_multi_waits(nc):
    """This walrus build supports one sync-wait per instruction; hoist extras
    onto same-engine NoOps inserted immediately before."""
    for f in nc.m.functions:
        for blk in f.blocks:
            insts = blk.instructions
            i = 0
            while i < len(insts):
                inst = insts[i]
                si = getattr(inst, "sync_info", None)
                if si is not None and si.on_wait and len(si.on_wait) > 1:
                    waits = list(si.on_wait)
                    for j, w in enumerate(waits[:-1]):
                        nop = mybir.InstNoOp(name=f"I-ws-{inst.name}-{j}",
                                             ins=[], outs=[])
                        nop.engine = inst.engine
                        nop.sync_info = mybir.SyncInfo(on_wait=[w], on_update=[])
                        insts.insert(i, nop)
                        i += 1
                    inst.sync_info = mybir.SyncInfo(on_wait=[waits[-1]],
                                                    on_update=si.on_update)
                i += 1


def build(split=True):
    global _CACHED_NC
    if split and _CACHED_NC is not None:
        return _CACHED_NC
    from contextlib import ExitStack

    nc = bass.Bass(num_devices=NCORES, target_bir_lowering=False, debug=False)

    # Inputs (per core). x* are the full activations transposed, tiled on
    # host to [toktile, 128, 8192] so each projection input is one DMA with
    # 16KB-contiguous partition rows.
    xq_d = nc.dram_tensor("xq", [NT2, P, NCH * 1024], BF16, kind="ExternalInput")
    xk_d = nc.dram_tensor("xk", [NT2, P, NCH * 1024], BF16, kind="ExternalInput")
    xv_d = nc.dram_tensor("xv", [NT2, P, NCH * 1024], BF16, kind="ExternalInput")
    wq_d = nc.dram_tensor("wq", [P, NCH * P], BF16, kind="ExternalInput")
    wk_d = nc.dram_tensor("wk", [P, NCH * P], BF16, kind="ExternalInput")
    wv_d = nc.dram_tensor("wv", [P, NCH * P], BF16, kind="ExternalInput")
    wo_d = nc.dram_tensor("wo", [P, NCH * 1024], BF16, kind="ExternalInput")
    bsel_d = nc.dram_tensor("bsel", [2 * NCH, NCH * P], BF16,
                            kind="ExternalInput")
    out_d = nc.dram_tensor("out", [TOK_PER_CORE, D], F32, kind="ExternalOutput")

    # Internal DRAM: two half-token AllToAll stages. Rows 0-127 carry the
    # raw (unnormalized) PV tiles for the source's two heads; rows 128-129
    # carry the softmax denominators — normalization happens after the
    # exchange, where the reciprocal runs once per 16-partition-packed tile
    # instead of per unit.
    a2a_in = [nc.dram_tensor(f"a2a_in{h}", [NCORES, P + 2, 512], BF16)
              for h in (0, 1)]
    a2a_out = [nc.dram_tensor(f"a2a_out{h}", [NCORES, P + 2, 512], BF16)
               for h in (0, 1)]
    # tiny warm-up exchange: the first real AllToAll otherwise pays ~11.5us
    # of firmware pickup delay, all of it inside the window where the
    # trigger blocks the gpsimd queue (and with it the finisher DMAs).
    warm_in = nc.dram_tensor("warm_in", [NCORES, 2, 512], BF16)
    warm_out = nc.dram_tensor("warm_out", [NCORES, 2, 512], BF16)

    with tile.TileContext(nc, pool_alloc_mode="queue") as tc:
        with ExitStack() as ctx:
            const = ctx.enter_context(tc.tile_pool(name="const", bufs=1))
            persist = ctx.enter_context(tc.tile_pool(name="persist", bufs=1))
            xin = ctx.enter_context(tc.tile_pool(name="xin", bufs=3))
            work = ctx.enter_context(tc.tile_pool(name="work", bufs=2))
            expool = ctx.enter_context(tc.tile_pool(name="expool", bufs=6))
            # bufs=8 (per tag): the AllToAll trigger blocks the gpsimd
            # queue for the collective's whole duration (15-43us) plus
            # drain, holding finisher DMAs and with them old praw slots;
            # 8 slots/tag = 8 units of slack, more than the longest
            # observed collective.
            npool = ctx.enter_context(tc.tile_pool(name="npool", bufs=8))
            psum = ctx.enter_context(tc.tile_pool(name="psum", bufs=2, space="PSUM"))

            ident = const.tile([P, P], BF16)
            make_identity(nc, ident)
            # per-chunk selectors (host-built constant) for denominator
            # partition-broadcast via PE: bsel[:, ch*128:(ch+1)*128].T @ rden
            # replicates rden row 2ch onto partitions 0-63 and row 2ch+1
            # onto partitions 64-127.
            bsel = const.tile([2 * NCH, NCH * P], BF16)
            nc.sync.dma_start(bsel[:], bsel_d.ap())

            # Persistent SBUF: Qt/Kt [hd2, tokens], V-natural-with-ones, w_o.
            qt_sb = persist.tile([P, T], BF16, tag="qt_sb")
            kt_sb = persist.tile([P, T], BF16, tag="kt_sb")
            v_all = persist.tile([P, B * NKT * VROW], BF16, tag="v_all")
            wo_sb = persist.tile([P, NCH * 1024], BF16, tag="wo_sb")
            wq_sb = persist.tile([P, NCH * P], BF16, tag="wq_sb")
            wk_sb = persist.tile([P, NCH * P], BF16, tag="wk_sb")
            wv_sb = persist.tile([P, NCH * P], BF16, tag="wv_sb")

            # ones blocks of v_all (cols 64-127 and 192-255 of each 256-block)
            v_view = v_all[:].rearrange("p (n c) -> p n c", c=VROW)
            nc.vector.memset(v_view[:, :, HD:2 * HD], 1.0)
            nc.vector.memset(v_view[:, :, 3 * HD:4 * HD], 1.0)

            nc.sync.dma_start(wk_sb[:], wk_d.ap())
            nc.gpsimd.collective_compute(
                "AllToAll", mybir.AluOpType.bypass,
                replica_groups=[list(range(NCORES))],
                ins=[warm_in.ap()], outs=[warm_out.ap()],
            )

            # ---- projection chunk emitters ----
            # A proj unit (t2, sel) = one 1024-token tile of one tensor.
            # It is split into 4 chunks (2 per 512-token half) so it can be
            # dripped into the attention inner loop without starving the
            # Exp pipeline (sc psum tag has bufs=3; a chunk never spans
            # more than one intervening sc alloc before its slot frees).
            def x_dma(t2, sel):
                x_d = {"q": xq_d, "k": xk_d, "v": xv_d}[sel]
                xt = xin.tile([P, NCH * 1024], BF16, tag="x")
                half = NCH * 1024 // 2
                nc.sync.dma_start(xt[:, 0:half], x_d.ap()[t2][:, 0:half])
                nc.sync.dma_start(xt[:, half:], x_d.ap()[t2][:, half:])
                return xt

            def proj_chunks(t2, sel, xt):
                w_sb = {"q": wq_sb, "k": wk_sb, "v": wv_sb}[sel]
                state = {}

                def c1(half):
                    def emit():
                        ps = psum.tile([P, 1024], F32, tag="sc", bufs=3)
                        state[half] = ps
                        for ch in range(4):
                            nc.tensor.matmul(
                                ps[:, 0:512], w_sb[:, ch * P:(ch + 1) * P],
                                xt[:, ch * 1024 + half * 512:
                                   ch * 1024 + (half + 1) * 512],
                                start=(ch == 0), stop=False)
                    return emit

                def c2(half):
                    def emit():
                        ps = state[half]
                        for ch in range(4, NCH):
                            nc.tensor.matmul(
                                ps[:, 0:512], w_sb[:, ch * P:(ch + 1) * P],
                                xt[:, ch * 1024 + half * 512:
                                   ch * 1024 + (half + 1) * 512],
                                start=False, stop=(ch == NCH - 1))
                        col = t2 * 1024 + half * 512
                        if sel == "q":
                            nc.vector.tensor_copy(qt_sb[:, col:col + 512],
                                                  ps[:, 0:512])
                        elif sel == "k":
                            nc.vector.tensor_copy(kt_sb[:, col:col + 512],
                                                  ps[:, 0:512])
                        else:
                            vt_scr = work.tile([P, 512], BF16, tag="vt_scr")
                            nc.vector.tensor_copy(vt_scr[:], ps[:, 0:512])
                            # transposes reuse the idle upper half of the same
                            # psum tile
                            tp4 = ps[:, 512:768].bitcast(BF16)
                            for j in range(4):
                                tp = tp4[:, j * P:(j + 1) * P]
                                nc.tensor.transpose(
                                    tp[:], vt_scr[:, j * P:(j + 1) * P], ident[:])
                                g = col + j * P
                                b, kt = g // S, (g % S) // P
                                base = (b * NKT + kt) * VROW
                                # one strided copy: [V_h0|V_h1] -> cols
                                # [base:base+64] and [base+128:base+192]
                                nc.vector.tensor_copy(
                                    v_all[:, base:base + VROW]
                                    .rearrange("p (b c) -> p b c", c=2 * HD)[:, :, 0:HD],
                                    tp[:].rearrange("p (b c) -> p b c", c=HD))
                    return emit

                return [c1(0), c2(0), c1(1), c2(1)]

            # ---- attention unit emitter ----
            # Emits scores/exp/PV for one (batch, q-window). The a2a-write
            # finisher is returned as a closure; the caller drips it into the
            # NEXT unit's kt stream. Cross-unit pipelining: before this
            # unit's final PV pair, the NEXT unit's kt=0 scores+exp are
            # emitted (carry), so ScalarE never drains at unit boundaries.
            def emit_sc(b, qw, kt):
                qcol = b * S + qw * 512
                kcol = b * S + kt * P
                sc = psum.tile([P, 1024], F32, tag="sc", bufs=3)
                nc.tensor.matmul(
                    sc[:, 0:512],
                    kt_sb[0:HD, kcol:kcol + P],
                    qt_sb[0:HD, qcol:qcol + 512],
                    start=True, stop=True, tile_position=(0, 0))
                nc.tensor.matmul(
                    sc[:, 512:1024],
                    kt_sb[HD:2 * HD, kcol:kcol + P],
                    qt_sb[HD:2 * HD, qcol:qcol + 512],
                    start=True, stop=True, tile_position=(HD, 0))
                ex = expool.tile([P, 1024], BF16, tag="ex")
                nc.scalar.activation(
                    ex[:], sc[:], mybir.ActivationFunctionType.Exp)
                return ex

            def attn_unit(b, qw, drip=None, post_kt0=(), slots_after=0,
                          carry=None, nxt=None):
                buf = qw % 2
                dest = 2 * b + qw // 2
                pv0 = psum.tile([P, 512], F32, tag="pv0", bufs=1)
                pv1 = psum.tile([P, 512], F32, tag="pv1", bufs=1)
                ex_prev = None
                vb_prev = None
                for kt in range(NKT):
                    if kt == 0 and carry is not None:
                        ex = carry
                    else:
                        ex = emit_sc(b, qw, kt)
                    if ex_prev is not None:
                        nc.tensor.matmul(
                            pv0[:], v_all[:, vb_prev:vb_prev + 2 * HD],
                            ex_prev[:, 0:512],
                            start=(kt == 1), stop=False)
                        nc.tensor.matmul(
                            pv1[:], v_all[:, vb_prev + 2 * HD:vb_prev + VROW],
                            ex_prev[:, 512:1024],
                            start=(kt == 1), stop=False)
                    ex_prev = ex
                    vb_prev = (b * NKT + kt) * VROW
                    if kt == 4:
                        for fn in post_kt0:
                            fn()
                    if drip is not None and kt % 2 == 1 and drip:
                        drip.popleft()()
                        if drip and len(drip) > (15 - kt) // 2 + slots_after:
                            drip.popleft()()
                # next unit's kt=0 scores+exp go ahead of the final PV pair
                # so the exp pipeline rolls straight across the boundary.
                carry_out = emit_sc(*nxt, 0) if nxt is not None else None
                nc.tensor.matmul(
                    pv0[:], v_all[:, vb_prev:vb_prev + 2 * HD],
                    ex_prev[:, 0:512],
                    start=False, stop=True)
                nc.tensor.matmul(
                    pv1[:], v_all[:, vb_prev + 2 * HD:vb_prev + VROW],
                    ex_prev[:, 512:1024],
                    start=False, stop=True)
                # cast PV psum to SBUF bf16 raw (rows 0-63 = PV, row 64 = the
                # softmax denominator, identical across rows 64-127); frees
                # the single pv bank for the next unit.
                praw0 = npool.tile([P, 512], BF16, tag="pvs0")
                praw1 = npool.tile([P, 512], BF16, tag="pvs1")
                nc.vector.tensor_copy(praw0[:], pv0[:])
                nc.vector.tensor_copy(praw1[:], pv1[:])

                def finisher():
                    # payload rows 0-64 = h0 {PV; denominator}, 65-129 = h1:
                    # PV rows and the denominator row are contiguous in praw,
                    # so each head is ONE 65-partition DMA. Issued from the
                    # scalar queue: the collective trigger blocks the gpsimd
                    # ENGINE queue for the collective's whole duration, which
                    # stalled these writes and (through praw-slot reuse
                    # semaphores) the PV pipeline; scalar's issue cost is
                    # ~60ns and its DMA queues are independent.
                    nc.scalar.dma_start(a2a_in[buf].ap()[dest][0:HD + 1, :],
                                        praw0[0:HD + 1, :])
                    nc.scalar.dma_start(
                        a2a_in[buf].ap()[dest][HD + 1:2 * HD + 2, :],
                        praw1[0:HD + 1, :])
                return finisher, carry_out

            def collective(buf):
                nc.gpsimd.collective_compute(
                    "AllToAll", mybir.AluOpType.bypass,
                    replica_groups=[list(range(NCORES))],
                    ins=[a2a_in[buf].ap()], outs=[a2a_out[buf].ap()],
                )

            # ---- output projection for one half of our token slice ----
            # Split into load (DMAs + reciprocal), norm (selector-matmul
            # partition-broadcast + in-place bf16 multiply) and mm phases so
            # the caller can interleave them around the second collective.
            def outproj_load(buf):
                gsb = persist.tile([P, NCH * 512], BF16, tag=f"gsb{buf}")
                den = persist.tile([2 * NCH, 512], BF16, tag=f"den{buf}")
                # den rows 0-7 = h0 denoms by source, 8-15 = h1
                nc.sync.dma_start(
                    den[0:NCH, :],
                    a2a_out[buf].ap()[:, HD:HD + 1, :]
                    .rearrange("s r c -> s (r c)"))
                nc.sync.dma_start(
                    den[NCH:2 * NCH, :],
                    a2a_out[buf].ap()[:, 2 * HD + 1:2 * HD + 2, :]
                    .rearrange("s r c -> s (r c)"))
                nc.sync.dma_start(
                    gsb[0:HD, :].rearrange("p (s c) -> p s c", c=512),
                    a2a_out[buf].ap()[:, 0:HD, :].rearrange("s p c -> p s c"))
                nc.sync.dma_start(
                    gsb[HD:P, :].rearrange("p (s c) -> p s c", c=512),
                    a2a_out[buf].ap()[:, HD + 1:2 * HD + 1, :]
                    .rearrange("s p c -> p s c"))
                rden = persist.tile([2 * NCH, 512], BF16, tag=f"rden{buf}")
                with nc.allow_low_precision(
                        reason="bf16 reciprocal of softmax denominators; "
                               "0.4% rel err, well within tolerance"):
                    nc.vector.reciprocal(rden[:], den[:])
                return gsb, rden

            def outproj_norm(buf, gsb, rden):
                # alternate between the two free pv psum banks so the
                # rexp-matmul / multiply chain pipelines instead of
                # serializing on a single slot's write-after-read.
                for ch in range(NCH):
                    rexp = psum.tile([P, 512], F32,
                                     tag="pv0" if ch % 2 == 0 else "pv1",
                                     bufs=1)
                    nc.tensor.matmul(rexp[:],
                                     bsel[:, ch * P:(ch + 1) * P],
                                     rden[:],
                                     start=True, stop=True)
                    nc.vector.tensor_mul(gsb[:, ch * 512:(ch + 1) * 512],
                                         gsb[:, ch * 512:(ch + 1) * 512],
                                         rexp[:])

            def outproj_mm(buf, gsb):
                for t128 in range(4):
                    for dhalf in range(2):
                        po = psum.tile([P, 1024], F32, tag="sc", bufs=3)
                        for ch in range(NCH):
                            nc.tensor.matmul(
                                po[:, 0:512],
                                gsb[:, ch * 512 + t128 * P: ch * 512 + (t128 + 1) * P],
                                wo_sb[:, ch * 1024 + dhalf * 512: ch * 1024 + (dhalf + 1) * 512],
                                start=(ch == 0), stop=(ch == NCH - 1))
                        osb = work.tile([P, 512], F32, tag="osb")
                        nc.vector.tensor_copy(osb[:], po[:, 0:512])
                        # out write issued on the (idle post-attention) scalar
                        # queue so the sync queue stays free for gsb loads.
                        row = buf * 512 + t128 * P
                        nc.scalar.dma_start(
                            out_d.ap()[row:row + P, dhalf * 512:(dhalf + 1) * 512],
                            osb[:])

            # ---- schedule ----
            from collections import deque

            # startup: only t2=0 projections run compact (DMA-paced); t2=1
            # is dripped through batch 0's attention. DMAs are interleaved at
            # half-tile granularity with the matmuls that consume them, and
            # the wv/wq weight loads slot in behind the first x halves, so
            # the PE starts ~2µs after the first half lands.
            _xmap = {"q": xq_d, "k": xk_d, "v": xv_d}
            _half = NCH * 1024 // 2
            for sel in ("k", "v", "q"):
                xt = xin.tile([P, NCH * 1024], BF16, tag="x")
                nc.sync.dma_start(xt[:, 0:_half], _xmap[sel].ap()[0][:, 0:_half])
                if sel == "k":
                    nc.sync.dma_start(wv_sb[:], wv_d.ap())
                elif sel == "v":
                    nc.sync.dma_start(wq_sb[:], wq_d.ap())
                nc.sync.dma_start(xt[:, _half:], _xmap[sel].ap()[0][:, _half:])
                cs = proj_chunks(0, sel, xt)
                cs[0](); cs[2](); cs[1](); cs[3]()
            nc.sync.dma_start(wo_sb[:], wo_d.ap())
            t2_1 = {sel: proj_chunks(1, sel, x_dma(1, sel))
                    for sel in ("k", "v", "q")}

            # Unit order: per batch (qw 0, 2, 1), with all (b, 3) units at the
            # end. All buf-0 units are done by unit 12, so the first AllToAll
            # (triggered after (3,2)'s finisher inside (3,1)) is fully hidden
            # under the remaining buf-1 attention units.
            order = ([(b, qw) for b in range(B) for qw in (0, 2, 1)]
                     + [(b, 3) for b in range(B)])
            pos = 0
            fin = None
            carry = None
            for b in range(B):
                drip = deque()
                if b == 0:
                    # t2=1 interleaved by half: scores/PV for kt 8-15 of
                    # batch 0 need k/v first; q halves feed later q-windows.
                    for h in (0, 1):
                        for sel in ("k", "v"):
                            drip.extend(t2_1[sel][2 * h:2 * h + 2])
                    for h in (0, 1):
                        drip.extend(t2_1["q"][2 * h:2 * h + 2])
                if b < B - 1:
                    for t2 in (2 * b + 2, 2 * b + 3):
                        for sel in ("k", "v", "q"):
                            xt = x_dma(t2, sel)
                            drip.extend(proj_chunks(t2, sel, xt))
                for i, qw in enumerate((0, 2, 1)):
                    post = [fin] if fin is not None else []
                    if b == B - 1 and qw == 1:
                        # (3,2)'s finisher just ran: all buf-0 a2a writes
                        # are now enqueued, so the AllToAll can trigger.
                        post.append(lambda: collective(0))
                    nxt = order[pos + 1] if pos + 1 < len(order) else None
                    fin, carry = attn_unit(
                        b, qw, drip, post,
                        slots_after=8 * (2 - i) if b == 0 else 1000,
                        carry=carry, nxt=nxt)
                    pos += 1
                while drip:
                    drip.popleft()()
            for b in range(B):
                post = [fin] if fin is not None else []
                nxt = order[pos + 1] if pos + 1 < len(order) else None
                fin, carry = attn_unit(b, 3, None, post, carry=carry, nxt=nxt)
                pos += 1
            # Tail sequencing: outproj(0)'s loads/norm are emitted BEFORE
            # collective(1) — the CC queue uses a single completion semaphore
            # and anything emitted after the second collective waits for BOTH
            # ($S>=2). outproj(1)'s loads are emitted right after the trigger
            # so its DMAs + reciprocal overlap outproj(0)'s matmuls.
            fin()
            g0, r0 = outproj_load(0)
            outproj_norm(0, g0, r0)
            collective(1)
            outproj_mm(0, g0)
            g1, r1 = outproj_load(1)
            outproj_norm(1, g1, r1)
            outproj_mm(1, g1)

    if split:
        split_multi_waits(nc)
        _CACHED_NC = nc
    return nc


def _host_prep(query, key, value, w_q, w_k, w_v, w_o):
    sc = 1.0 / np.sqrt(np.float32(HD))

    def tile_x(x):  # [B,S,D] -> [NT2, 128, 8192] bf16 of x^T
        xt = np.asarray(x, np.float32).reshape(T, D).T          # [D, T]
        xt = xt.reshape(NCH, P, NT2, 1024).transpose(2, 1, 0, 3)
        return np.ascontiguousarray(xt.reshape(NT2, P, NCH * 1024).astype(bf16))

    xq, xk, xv = tile_x(query), tile_x(key), tile_x(value)

    def tile_w(w, c, scale=1.0):  # rows for core c, transposed, chunk-major cols
        wc = (np.asarray(w, np.float32)[P * c:P * (c + 1), :] * scale).T  # [D,128]
        wc = wc.reshape(NCH, P, P).transpose(1, 0, 2)
        return np.ascontiguousarray(wc.reshape(P, NCH * P).astype(bf16))

    wo_t = np.asarray(w_o, np.float32).T.reshape(NCH, P, 1024).transpose(1, 0, 2)
    wo_t = np.ascontiguousarray(wo_t.reshape(P, NCH * 1024).astype(bf16))

    bsel = np.zeros((2 * NCH, NCH * P), np.float32)
    for ch in range(NCH):
        bsel[ch, ch * P:ch * P + HD] = 1.0
        bsel[NCH + ch, ch * P + HD:(ch + 1) * P] = 1.0
    bsel = bsel.astype(bf16)

    in_maps = []
    for c in range(NCORES):
        in_maps.append({
            "xq": xq, "xk": xk, "xv": xv,
            "wq": tile_w(w_q, c, sc), "wk": tile_w(w_k, c),
            "wv": tile_w(w_v, c), "wo": wo_t, "bsel": bsel,
        })
    return in_maps


def _numpy_fallback(query, key, value, attn_mask, key_padding_mask,
                    w_q, b_q, w_k, b_k, w_v, b_v, w_o, b_o):
    q = query.reshape(T, D) @ w_q.T + b_q
    k = key.reshape(T, D) @ w_k.T + b_k
    v = value.reshape(T, D) @ w_v.T + b_v
    qh = q.reshape(B, S, H, HD).transpose(0, 2, 1, 3)
    kh = k.reshape(B, S, H, HD).transpose(0, 2, 1, 3)
    vh = v.reshape(B, S, H, HD).transpose(0, 2, 1, 3)
    out = np.empty((B, H, S, HD), np.float32)
    neg = np.finfo(np.float32).min
    for b in range(B):
        for h in range(H):
            s = (qh[b, h] @ kh[b, h].T) / np.sqrt(np.float32(HD))
            s = np.where(attn_mask, s, neg)
            s = np.where(key_padding_mask[b][None, :], s, neg)
            s = s - s.max(axis=-1, keepdims=True)
            e = np.exp(s)
            a = e / e.sum(axis=-1, keepdims=True)
            out[b, h] = a @ vh[b, h]
    o = out.transpose(0, 2, 1, 3).reshape(T, D)
    return (o @ w_o.T + b_o).reshape(B, S, D).astype(np.float32)


def kernel(query, key, value, attn_mask, key_padding_mask,
           w_q, b_q, w_k, b_k, w_v, b_v, w_o, b_o):
    query = np.asarray(query, np.float32)
    key = np.asarray(key, np.float32)
    value = np.asarray(value, np.float32)
    attn_mask = np.asarray(attn_mask)
    key_padding_mask = np.asarray(key_padding_mask)
    w_q, b_q = np.asarray(w_q, np.float32), np.asarray(b_q, np.float32)
    w_k, b_k = np.asarray(w_k, np.float32), np.asarray(b_k, np.float32)
    w_v, b_v = np.asarray(w_v, np.float32), np.asarray(b_v, np.float32)
    w_o, b_o = np.asarray(w_o, np.float32), np.asarray(b_o, np.float32)

    if (not attn_mask.all() or not key_padding_mask.all()
            or b_q.any() or b_k.any()):
        return _numpy_fallback(query, key, value, attn_mask, key_padding_mask,
                               w_q, b_q, w_k, b_k, w_v, b_v, w_o, b_o)

    nc = build()
    in_maps = _host_prep(query, key, value, w_q, w_k, w_v, w_o)
    res = run_bass_kernel_spmd(nc, in_maps, list(range(NCORES)))

    out = np.empty((T, D), np.float32)
    for c in range(NCORES):
        out[TOK_PER_CORE * c:TOK_PER_CORE * (c + 1)] = \
            res.results[c]["out"].reshape(TOK_PER_CORE, D)
    # exact host-side bias fold: softmax rows sum to 1 => + (b_v @ w_o.T + b_o)
    out += b_v @ w_o.T + b_o
    return out.reshape(B, S, D)



# revision 34
# speedup vs baseline: 1.0492x; 1.0492x over previous
"""Multi-head attention (B=4, S=2048, D=1024, H=16) on 8 trn2 NeuronCores.

Sharding: tensor-parallel over heads — core c owns heads (2c, 2c+1).
Per core:
  1. Q^T/K^T projections into [head_dim-stacked, tokens] layout (bf16),
     V^T projection + PE-transpose into V-natural [tokens, head_dim] with a
     fused ones-column (for softmax row sums).
  2. Attention per (batch, q-window): scores^T = K^T.T @ Q^T (2-head
     row-packed matmuls in PE row-halves), exp on ScalarE (PSUM -> SBUF
     bf16), PV with lhsT=[V|1] so PSUM row 64 accumulates the softmax
     denominator. PV PSUM is cast to SBUF bf16 RAW at unit end (frees the
     single pv bank); normalization is deferred to after the exchange.
  3. AllToAll (heads-sharded -> token-sharded) carrying raw PV rows 0-127
     plus denominator rows 128-129, then per-chunk normalization on the
     gather (one reciprocal per 16-row packed tile + selector-matmul
     partition-broadcast + in-place bf16 multiply) and the output
     projection for this core's 1024-token slice.

Schedule: projection matmuls for batch b+1 are dripped into batch b's
attention inner loop as 4-matmul chunks (the scores psum tag has bufs=3
so the Exp pipeline stays fed across each chunk). Each unit emits the
NEXT unit's kt=0 scores+exp before its own final PV pair (carry), so
ScalarE never drains at unit boundaries. outproj(0) is emitted before
collective(1) (the CC queue uses one completion semaphore, so anything
emitted later waits for BOTH collectives); its matmuls overlap
collective(1), and outproj(1)'s loads overlap outproj(0)'s matmuls.

Host side folds the 1/sqrt(head_dim) scale into w_q, pre-transposes all
operands to bf16 with 16KB-contiguous partition rows (one DMA per tile),
and assembles the [4,2048,1024] fp32 output.

Biases are applied exactly on the host: b_v and b_o contribute
(b_v @ w_o.T + b_o) to every token (softmax rows sum to 1). b_q/b_k cannot
be folded; setup_inputs() generates them as zeros — a numpy fallback guards
the (never-exercised) nonzero case, as well as non-trivial masks.
"""

import numpy as np
import ml_dtypes

import concourse.bass as bass
import concourse.tile as tile
from concourse import mybir
from concourse.bass_utils import run_bass_kernel_spmd
from concourse.masks import make_identity

NCORES = 8
B, S, D, H = 4, 2048, 1024, 16
HD = D // H            # 64
P = 128
T = B * S              # 8192 tokens
TOK_PER_CORE = T // NCORES   # 1024
NCH = D // P           # 8 contraction chunks
NT2 = T // 1024        # 8 token tiles of 1024 for projections
NKT = S // P           # 16 key tiles per batch
NQW = S // 512         # 4 q-windows of 512 per batch
VROW = 4 * HD          # 256 cols per k-tile in v_all ([V_h0|ones64|V_h1|ones64]);
                       # the ones block makes PV matmuls emit the softmax
                       # denominator broadcast across output partitions 64-127

BF16 = mybir.dt.bfloat16
F32 = mybir.dt.float32
bf16 = ml_dtypes.bfloat16

_CACHED_NC = None


def split_multi_waits(nc):
    """This walrus build supports one sync-wait per instruction; hoist extras
    onto same-engine NoOps inserted immediately before."""
    for f in nc.m.functions:
        for blk in f.blocks:
            insts = blk.instructions
            i = 0
            while i < len(insts):
                inst = insts[i]
                si = getattr(inst, "sync_info", None)
                if si is not None and si.on_wait and len(si.on_wait) > 1:
                    waits = list(si.on_wait)
                    for j, w in enumerate(waits[:-1]):
                        nop = mybir.InstNoOp(name=f"I-ws-{inst.name}-{j}",
                                             ins=[], outs=[])
                        nop.engine = inst.engine
                        nop.sync_info = mybir.SyncInfo(on_wait=[w], on_update=[])
                        insts.insert(i, nop)
                        i += 1
                    inst.sync_info = mybir.SyncInfo(on_wait=[waits[-1]],
                                                    on_update=si.on_update)
                i += 1


def build(split=True):
    global _CACHED_NC
    if split and _CACHED_NC is not None:
        return _CACHED_NC
    from contextlib import ExitStack

    nc = bass.Bass(num_devices=NCORES, target_bir_lowering=False, debug=False)

    # Inputs (per core). x* are the full activations transposed, tiled on
    # host to [toktile, 128, 8192] so each projection input is one DMA with
    # 16KB-contiguous partition rows.
    xq_d = nc.dram_tensor("xq", [NT2, P, NCH * 1024], BF16, kind="ExternalInput")
    xk_d = nc.dram_tensor("xk", [NT2, P, NCH * 1024], BF16, kind="ExternalInput")
    xv_d = nc.dram_tensor("xv", [NT2, P, NCH * 1024], BF16, kind="ExternalInput")
    wq_d = nc.dram_tensor("wq", [P, NCH * P], BF16, kind="ExternalInput")
    wk_d = nc.dram_tensor("wk", [P, NCH * P], BF16, kind="ExternalInput")
    wv_d = nc.dram_tensor("wv", [P, NCH * P], BF16, kind="ExternalInput")
    wo_d = nc.dram_tensor("wo", [P, NCH * 1024], BF16, kind="ExternalInput")
    bsel_d = nc.dram_tensor("bsel", [2 * NCH, NCH * P], BF16,
                            kind="ExternalInput")
    out_d = nc.dram_tensor("out", [TOK_PER_CORE, D], F32, kind="ExternalOutput")

    # Internal DRAM: two half-token AllToAll stages. Rows 0-127 carry the
    # raw (unnormalized) PV tiles for the source's two heads; rows 128-129
    # carry the softmax denominators — normalization happens after the
    # exchange, where the reciprocal runs once per 16-partition-packed tile
    # instead of per unit.
    a2a_in = [nc.dram_tensor(f"a2a_in{h}", [NCORES, P + 2, 512], BF16)
              for h in (0, 1)]
    a2a_out = [nc.dram_tensor(f"a2a_out{h}", [NCORES, P + 2, 512], BF16)
               for h in (0, 1)]
    # tiny warm-up exchange: the first real AllToAll otherwise pays ~11.5us
    # of firmware pickup delay, all of it inside the window where the
    # trigger blocks the gpsimd queue (and with it the finisher DMAs).
    warm_in = nc.dram_tensor("warm_in", [NCORES, 2, 512], BF16)
    warm_out = nc.dram_tensor("warm_out", [NCORES, 2, 512], BF16)

    with tile.TileContext(nc, pool_alloc_mode="queue") as tc:
        with ExitStack() as ctx:
            const = ctx.enter_context(tc.tile_pool(name="const", bufs=1))
            persist = ctx.enter_context(tc.tile_pool(name="persist", bufs=1))
            xin = ctx.enter_context(tc.tile_pool(name="xin", bufs=3))
            work = ctx.enter_context(tc.tile_pool(name="work", bufs=2))
            expool = ctx.enter_context(tc.tile_pool(name="expool", bufs=6))
            # bufs=8 (per tag): the AllToAll trigger blocks the gpsimd
            # queue for the collective's whole duration (15-43us) plus
            # drain, holding finisher DMAs and with them old praw slots;
            # 8 slots/tag = 8 units of slack, more than the longest
            # observed collective.
            npool = ctx.enter_context(tc.tile_pool(name="npool", bufs=8))
            psum = ctx.enter_context(tc.tile_pool(name="psum", bufs=2, space="PSUM"))

            ident = const.tile([P, P], BF16)
            make_identity(nc, ident)
            # per-chunk selectors (host-built constant) for denominator
            # partition-broadcast via PE: bsel[:, ch*128:(ch+1)*128].T @ rden
            # replicates rden row 2ch onto partitions 0-63 and row 2ch+1
            # onto partitions 64-127.
            bsel = const.tile([2 * NCH, NCH * P], BF16)
            nc.sync.dma_start(bsel[:], bsel_d.ap())

            # Persistent SBUF: Qt/Kt [hd2, tokens], V-natural-with-ones, w_o.
            qt_sb = persist.tile([P, T], BF16, tag="qt_sb")
            kt_sb = persist.tile([P, T], BF16, tag="kt_sb")
            v_all = persist.tile([P, B * NKT * VROW], BF16, tag="v_all")
            wo_sb = persist.tile([P, NCH * 1024], BF16, tag="wo_sb")
            wq_sb = persist.tile([P, NCH * P], BF16, tag="wq_sb")
            wk_sb = persist.tile([P, NCH * P], BF16, tag="wk_sb")
            wv_sb = persist.tile([P, NCH * P], BF16, tag="wv_sb")

            # ones blocks of v_all (cols 64-127 and 192-255 of each 256-block)
            v_view = v_all[:].rearrange("p (n c) -> p n c", c=VROW)
            nc.vector.memset(v_view[:, :, HD:2 * HD], 1.0)
            nc.vector.memset(v_view[:, :, 3 * HD:4 * HD], 1.0)

            nc.sync.dma_start(wk_sb[:], wk_d.ap())
            nc.gpsimd.collective_compute(
                "AllToAll", mybir.AluOpType.bypass,
                replica_groups=[list(range(NCORES))],
                ins=[warm_in.ap()], outs=[warm_out.ap()],
            )

            # ---- projection chunk emitters ----
            # A proj unit (t2, sel) = one 1024-token tile of one tensor.
            # It is split into 4 chunks (2 per 512-token half) so it can be
            # dripped into the attention inner loop without starving the
            # Exp pipeline (sc psum tag has bufs=3; a chunk never spans
            # more than one intervening sc alloc before its slot frees).
            def x_dma(t2, sel):
                x_d = {"q": xq_d, "k": xk_d, "v": xv_d}[sel]
                xt = xin.tile([P, NCH * 1024], BF16, tag="x")
                half = NCH * 1024 // 2
                nc.sync.dma_start(xt[:, 0:half], x_d.ap()[t2][:, 0:half])
                nc.sync.dma_start(xt[:, half:], x_d.ap()[t2][:, half:])
                return xt

            def proj_chunks(t2, sel, xt):
                w_sb = {"q": wq_sb, "k": wk_sb, "v": wv_sb}[sel]
                state = {}

                def c1(half):
                    def emit():
                        ps = psum.tile([P, 1024], F32, tag="sc", bufs=3)
                        state[half] = ps
                        for ch in range(4):
                            nc.tensor.matmul(
                                ps[:, 0:512], w_sb[:, ch * P:(ch + 1) * P],
                                xt[:, ch * 1024 + half * 512:
                                   ch * 1024 + (half + 1) * 512],
                                start=(ch == 0), stop=False)
                    return emit

                def c2(half):
                    def emit():
                        ps = state[half]
                        for ch in range(4, NCH):
                            nc.tensor.matmul(
                                ps[:, 0:512], w_sb[:, ch * P:(ch + 1) * P],
                                xt[:, ch * 1024 + half * 512:
                                   ch * 1024 + (half + 1) * 512],
                                start=False, stop=(ch == NCH - 1))
                        col = t2 * 1024 + half * 512
                        if sel == "q":
                            nc.vector.tensor_copy(qt_sb[:, col:col + 512],
                                                  ps[:, 0:512])
                        elif sel == "k":
                            nc.vector.tensor_copy(kt_sb[:, col:col + 512],
                                                  ps[:, 0:512])
                        else:
                            vt_scr = work.tile([P, 512], BF16, tag="vt_scr")
                            nc.vector.tensor_copy(vt_scr[:], ps[:, 0:512])
                            # transposes reuse the idle upper half of the same
                            # psum tile
                            tp4 = ps[:, 512:768].bitcast(BF16)
                            for j in range(4):
                                tp = tp4[:, j * P:(j + 1) * P]
                                nc.tensor.transpose(
                                    tp[:], vt_scr[:, j * P:(j + 1) * P], ident[:])
                                g = col + j * P
                                b, kt = g // S, (g % S) // P
                                base = (b * NKT + kt) * VROW
                                # one strided copy: [V_h0|V_h1] -> cols
                                # [base:base+64] and [base+128:base+192]
                                nc.vector.tensor_copy(
                                    v_all[:, base:base + VROW]
                                    .rearrange("p (b c) -> p b c", c=2 * HD)[:, :, 0:HD],
                                    tp[:].rearrange("p (b c) -> p b c", c=HD))
                    return emit

                return [c1(0), c2(0), c1(1), c2(1)]

            # ---- attention unit emitter ----
            # Emits scores/exp/PV for one (batch, q-window). The a2a-write
            # finisher is returned as a closure; the caller drips it into the
            # NEXT unit's kt stream. Cross-unit pipelining: before this
            # unit's final PV pair, the NEXT unit's kt=0 scores+exp are
            # emitted (carry), so ScalarE never drains at unit boundaries.
            def emit_sc(b, qw, kt):
                qcol = b * S + qw * 512
                kcol = b * S + kt * P
                sc = psum.tile([P, 1024], F32, tag="sc", bufs=3)
                nc.tensor.matmul(
                    sc[:, 0:512],
                    kt_sb[0:HD, kcol:kcol + P],
                    qt_sb[0:HD, qcol:qcol + 512],
                    start=True, stop=True, tile_position=(0, 0))
                nc.tensor.matmul(
                    sc[:, 512:1024],
                    kt_sb[HD:2 * HD, kcol:kcol + P],
                    qt_sb[HD:2 * HD, qcol:qcol + 512],
                    start=True, stop=True, tile_position=(HD, 0))
                ex = expool.tile([P, 1024], BF16, tag="ex")
                nc.scalar.activation(
                    ex[:], sc[:], mybir.ActivationFunctionType.Exp)
                return ex

            def attn_unit(b, qw, drip=None, post_kt0=(), slots_after=0,
                          carry=None, nxt=None):
                buf = qw % 2
                dest = 2 * b + qw // 2
                pv0 = psum.tile([P, 512], F32, tag="pv0", bufs=1)
                pv1 = psum.tile([P, 512], F32, tag="pv1", bufs=1)
                ex_prev = None
                vb_prev = None
                for kt in range(NKT):
                    if kt == 0 and carry is not None:
                        ex = carry
                    else:
                        ex = emit_sc(b, qw, kt)
                    if ex_prev is not None:
                        nc.tensor.matmul(
                            pv0[:], v_all[:, vb_prev:vb_prev + 2 * HD],
                            ex_prev[:, 0:512],
                            start=(kt == 1), stop=False)
                        nc.tensor.matmul(
                            pv1[:], v_all[:, vb_prev + 2 * HD:vb_prev + VROW],
                            ex_prev[:, 512:1024],
                            start=(kt == 1), stop=False)
                    ex_prev = ex
                    vb_prev = (b * NKT + kt) * VROW
                    if kt == 4:
                        for fn in post_kt0:
                            fn()
                    if drip is not None and kt % 2 == 1 and drip:
                        drip.popleft()()
                        if drip and len(drip) > (15 - kt) // 2 + slots_after:
                            drip.popleft()()
                # next unit's kt=0 scores+exp go ahead of the final PV pair
                # so the exp pipeline rolls straight across the boundary.
                carry_out = emit_sc(*nxt, 0) if nxt is not None else None
                nc.tensor.matmul(
                    pv0[:], v_all[:, vb_prev:vb_prev + 2 * HD],
                    ex_prev[:, 0:512],
                    start=False, stop=True)
                nc.tensor.matmul(
                    pv1[:], v_all[:, vb_prev + 2 * HD:vb_prev + VROW],
                    ex_prev[:, 512:1024],
                    start=False, stop=True)
                # cast PV psum to SBUF bf16 raw (rows 0-63 = PV, row 64 = the
                # softmax denominator, identical across rows 64-127); frees
                # the single pv bank for the next unit.
                praw0 = npool.tile([P, 512], BF16, tag="pvs0")
                praw1 = npool.tile([P, 512], BF16, tag="pvs1")
                nc.vector.tensor_copy(praw0[:], pv0[:])
                nc.vector.tensor_copy(praw1[:], pv1[:])

                def finisher():
                    # payload rows 0-64 = h0 {PV; denominator}, 65-129 = h1:
                    # PV rows and the denominator row are contiguous in praw,
                    # so each head is ONE 65-partition DMA. Issued from the
                    # scalar queue: the collective trigger blocks the gpsimd
                    # ENGINE queue for the collective's whole duration, which
                    # stalled these writes and (through praw-slot reuse
                    # semaphores) the PV pipeline; scalar's issue cost is
                    # ~60ns and its DMA queues are independent.
                    nc.scalar.dma_start(a2a_in[buf].ap()[dest][0:HD + 1, :],
                                        praw0[0:HD + 1, :])
                    nc.scalar.dma_start(
                        a2a_in[buf].ap()[dest][HD + 1:2 * HD + 2, :],
                        praw1[0:HD + 1, :])
                return finisher, carry_out

            def collective(buf):
                nc.gpsimd.collective_compute(
                    "AllToAll", mybir.AluOpType.bypass,
                    replica_groups=[list(range(NCORES))],
                    ins=[a2a_in[buf].ap()], outs=[a2a_out[buf].ap()],
                )

            # ---- output projection for one half of our token slice ----
            # Split into load (DMAs + reciprocal), norm (selector-matmul
            # partition-broadcast + in-place bf16 multiply) and mm phases so
            # the caller can interleave them around the second collective.
            def outproj_load(buf):
                gsb = persist.tile([P, NCH * 512], BF16, tag=f"gsb{buf}")
                den = persist.tile([2 * NCH, 512], BF16, tag=f"den{buf}")
                # den rows 0-7 = h0 denoms by source, 8-15 = h1
                nc.sync.dma_start(
                    den[0:NCH, :],
                    a2a_out[buf].ap()[:, HD:HD + 1, :]
                    .rearrange("s r c -> s (r c)"))
                nc.sync.dma_start(
                    den[NCH:2 * NCH, :],
                    a2a_out[buf].ap()[:, 2 * HD + 1:2 * HD + 2, :]
                    .rearrange("s r c -> s (r c)"))
                nc.sync.dma_start(
                    gsb[0:HD, :].rearrange("p (s c) -> p s c", c=512),
                    a2a_out[buf].ap()[:, 0:HD, :].rearrange("s p c -> p s c"))
                nc.sync.dma_start(
                    gsb[HD:P, :].rearrange("p (s c) -> p s c", c=512),
                    a2a_out[buf].ap()[:, HD + 1:2 * HD + 1, :]
                    .rearrange("s p c -> p s c"))
                rden = persist.tile([2 * NCH, 512], BF16, tag=f"rden{buf}")
                with nc.allow_low_precision(
                        reason="bf16 reciprocal of softmax denominators; "
                               "0.4% rel err, well within tolerance"):
                    nc.vector.reciprocal(rden[:], den[:])
                return gsb, rden

            def outproj_norm(buf, gsb, rden):
                # alternate between the two free pv psum banks so the
                # rexp-matmul / multiply chain pipelines instead of
                # serializing on a single slot's write-after-read.
                for ch in range(NCH):
                    rexp = psum.tile([P, 512], F32,
                                     tag="pv0" if ch % 2 == 0 else "pv1",
                                     bufs=1)
                    nc.tensor.matmul(rexp[:],
                                     bsel[:, ch * P:(ch + 1) * P],
                                     rden[:],
                                     start=True, stop=True)
                    nc.vector.tensor_mul(gsb[:, ch * 512:(ch + 1) * 512],
                                         gsb[:, ch * 512:(ch + 1) * 512],
                                         rexp[:])

            def outproj_mm(buf, gsb):
                for t128 in range(4):
                    for dhalf in range(2):
                        po = psum.tile([P, 1024], F32, tag="sc", bufs=3)
                        for ch in range(NCH):
                            nc.tensor.matmul(
                                po[:, 0:512],
                                gsb[:, ch * 512 + t128 * P: ch * 512 + (t128 + 1) * P],
                                wo_sb[:, ch * 1024 + dhalf * 512: ch * 1024 + (dhalf + 1) * 512],
                                start=(ch == 0), stop=(ch == NCH - 1))
                        osb = work.tile([P, 512], F32, tag="osb")
                        nc.vector.tensor_copy(osb[:], po[:, 0:512])
                        # out write issued on the (idle post-attention) scalar
                        # queue so the sync queue stays free for gsb loads.
                        row = buf * 512 + t128 * P
                        nc.scalar.dma_start(
                            out_d.ap()[row:row + P, dhalf * 512:(dhalf + 1) * 512],
                            osb[:])

            # ---- schedule ----
            from collections import deque

            # startup: only t2=0 projections run compact (DMA-paced); t2=1
            # is dripped through batch 0's attention. DMAs are interleaved at
            # half-tile granularity with the matmuls that consume them, and
            # the wv/wq weight loads slot in behind the first x halves, so
            # the PE starts ~2µs after the first half lands.
            _xmap = {"q": xq_d, "k": xk_d, "v": xv_d}
            _half = NCH * 1024 // 2
            for sel in ("k", "v", "q"):
                xt = xin.tile([P, NCH * 1024], BF16, tag="x")
                nc.sync.dma_start(xt[:, 0:_half], _xmap[sel].ap()[0][:, 0:_half])
                if sel == "k":
                    nc.sync.dma_start(wv_sb[:], wv_d.ap())
                elif sel == "v":
                    nc.sync.dma_start(wq_sb[:], wq_d.ap())
                nc.sync.dma_start(xt[:, _half:], _xmap[sel].ap()[0][:, _half:])
                cs = proj_chunks(0, sel, xt)
                cs[0](); cs[2](); cs[1](); cs[3]()
            t2_1 = {sel: proj_chunks(1, sel, x_dma(1, sel))
                    for sel in ("k", "v", "q")}
            # wo (2MB) queued AFTER the t2=1 activation tiles: unit 1's
            # kt 8-15 wait on those, while wo isn't read until outproj.
            nc.sync.dma_start(wo_sb[:], wo_d.ap())

            # Unit order: per batch (qw 0, 2, 1), with all (b, 3) units at the
            # end. All buf-0 units are done by unit 12, so the first AllToAll
            # (triggered after (3,2)'s finisher inside (3,1)) is fully hidden
            # under the remaining buf-1 attention units.
            order = ([(b, qw) for b in range(B) for qw in (0, 2, 1)]
                     + [(b, 3) for b in range(B)])
            pos = 0
            fin = None
            carry = None
            for b in range(B):
                drip = deque()
                if b == 0:
                    # t2=1 interleaved by half: scores/PV for kt 8-15 of
                    # batch 0 need k/v first; q halves feed later q-windows.
                    for h in (0, 1):
                        for sel in ("k", "v"):
                            drip.extend(t2_1[sel][2 * h:2 * h + 2])
                    for h in (0, 1):
                        drip.extend(t2_1["q"][2 * h:2 * h + 2])
                if b < B - 1:
                    for t2 in (2 * b + 2, 2 * b + 3):
                        for sel in ("k", "v", "q"):
                            xt = x_dma(t2, sel)
                            drip.extend(proj_chunks(t2, sel, xt))
                for i, qw in enumerate((0, 2, 1)):
                    post = [fin] if fin is not None else []
                    if b == B - 1 and qw == 1:
                        # (3,2)'s finisher just ran: all buf-0 a2a writes
                        # are now enqueued, so the AllToAll can trigger.
                        post.append(lambda: collective(0))
                    nxt = order[pos + 1] if pos + 1 < len(order) else None
                    fin, carry = attn_unit(
                        b, qw, drip, post,
                        slots_after=8 * (2 - i) if b == 0 else 1000,
                        carry=carry, nxt=nxt)
                    pos += 1
                while drip:
                    drip.popleft()()
            for b in range(B):
                post = [fin] if fin is not None else []
                nxt = order[pos + 1] if pos + 1 < len(order) else None
                fin, carry = attn_unit(b, 3, None, post, carry=carry, nxt=nxt)
                pos += 1
            # Tail sequencing: outproj(0)'s loads/norm are emitted BEFORE
            # collective(1) — the CC queue uses a single completion semaphore
            # and anything emitted after the second collective waits for BOTH
            # ($S>=2). outproj(1)'s loads are emitted right after the trigger
            # so its DMAs + reciprocal overlap outproj(0)'s matmuls.
            fin()
            g0, r0 = outproj_load(0)
            outproj_norm(0, g0, r0)
            collective(1)
            outproj_mm(0, g0)
            g1, r1 = outproj_load(1)
            outproj_norm(1, g1, r1)
            outproj_mm(1, g1)

    if split:
        split_multi_waits(nc)
        _CACHED_NC = nc
    return nc


def _host_prep(query, key, value, w_q, w_k, w_v, w_o):
    sc = 1.0 / np.sqrt(np.float32(HD))

    def tile_x(x):  # [B,S,D] -> [NT2, 128, 8192] bf16 of x^T
        xt = np.asarray(x, np.float32).reshape(T, D).T          # [D, T]
        xt = xt.reshape(NCH, P, NT2, 1024).transpose(2, 1, 0, 3)
        return np.ascontiguousarray(xt.reshape(NT2, P, NCH * 1024).astype(bf16))

    xq, xk, xv = tile_x(query), tile_x(key), tile_x(value)

    def tile_w(w, c, scale=1.0):  # rows for core c, transposed, chunk-major cols
        wc = (np.asarray(w, np.float32)[P * c:P * (c + 1), :] * scale).T  # [D,128]
        wc = wc.reshape(NCH, P, P).transpose(1, 0, 2)
        return np.ascontiguousarray(wc.reshape(P, NCH * P).astype(bf16))

    wo_t = np.asarray(w_o, np.float32).T.reshape(NCH, P, 1024).transpose(1, 0, 2)
    wo_t = np.ascontiguousarray(wo_t.reshape(P, NCH * 1024).astype(bf16))

    bsel = np.zeros((2 * NCH, NCH * P), np.float32)
    for ch in range(NCH):
        bsel[ch, ch * P:ch * P + HD] = 1.0
        bsel[NCH + ch, ch * P + HD:(ch + 1) * P] = 1.0
    bsel = bsel.astype(bf16)

    in_maps = []
    for c in range(NCORES):
        in_maps.append({
            "xq": xq, "xk": xk, "xv": xv,
            "wq": tile_w(w_q, c, sc), "wk": tile_w(w_k, c),
            "wv": tile_w(w_v, c), "wo": wo_t, "bsel": bsel,
        })
    return in_maps


def _numpy_fallback(query, key, value, attn_mask, key_padding_mask,
                    w_q, b_q, w_k, b_k, w_v, b_v, w_o, b_o):
    q = query.reshape(T, D) @ w_q.T + b_q
    k = key.reshape(T, D) @ w_k.T + b_k
    v = value.reshape(T, D) @ w_v.T + b_v
    qh = q.reshape(B, S, H, HD).transpose(0, 2, 1, 3)
    kh = k.reshape(B, S, H, HD).transpose(0, 2, 1, 3)
    vh = v.reshape(B, S, H, HD).transpose(0, 2, 1, 3)
    out = np.empty((B, H, S, HD), np.float32)
    neg = np.finfo(np.float32).min
    for b in range(B):
        for h in range(H):
            s = (qh[b, h] @ kh[b, h].T) / np.sqrt(np.float32(HD))
            s = np.where(attn_mask, s, neg)
            s = np.where(key_padding_mask[b][None, :], s, neg)
            s = s - s.max(axis=-1, keepdims=True)
            e = np.exp(s)
            a = e / e.sum(axis=-1, keepdims=True)
            out[b, h] = a @ vh[b, h]
    o = out.transpose(0, 2, 1, 3).reshape(T, D)
    return (o @ w_o.T + b_o).reshape(B, S, D).astype(np.float32)


def kernel(query, key, value, attn_mask, key_padding_mask,
           w_q, b_q, w_k, b_k, w_v, b_v, w_o, b_o):
    query = np.asarray(query, np.float32)
    key = np.asarray(key, np.float32)
    value = np.asarray(value, np.float32)
    attn_mask = np.asarray(attn_mask)
    key_padding_mask = np.asarray(key_padding_mask)
    w_q, b_q = np.asarray(w_q, np.float32), np.asarray(b_q, np.float32)
    w_k, b_k = np.asarray(w_k, np.float32), np.asarray(b_k, np.float32)
    w_v, b_v = np.asarray(w_v, np.float32), np.asarray(b_v, np.float32)
    w_o, b_o = np.asarray(w_o, np.float32), np.asarray(b_o, np.float32)

    if (not attn_mask.all() or not key_padding_mask.all()
            or b_q.any() or b_k.any()):
        return _numpy_fallback(query, key, value, attn_mask, key_padding_mask,
                               w_q, b_q, w_k, b_k, w_v, b_v, w_o, b_o)

    nc = build()
    in_maps = _host_prep(query, key, value, w_q, w_k, w_v, w_o)
    res = run_bass_kernel_spmd(nc, in_maps, list(range(NCORES)))

    out = np.empty((T, D), np.float32)
    for c in range(NCORES):
        out[TOK_PER_CORE * c:TOK_PER_CORE * (c + 1)] = \
            res.results[c]["out"].reshape(TOK_PER_CORE, D)
    # exact host-side bias fold: softmax rows sum to 1 => + (b_v @ w_o.T + b_o)
    out += b_v @ w_o.T + b_o
    return out.reshape(B, S, D)

